# revision 26
# baseline (speedup 1.0000x reference)
"""DiffMoE MLP (8 experts, capacity 1.0) — expert-parallel across 8 TRN2 NeuronCores.

Contract: kernel(**full_inputs) -> full output (4, 2048, 1024) f32.

Strategy (expert-parallel per sharding_hint):
  host   : gating scores + per-expert top-k (bit-identical jnp ops to the
           reference), token gather + fp32 LayerNorm, fp8 hi/lo weight
           decomposition + re-layout, and the final bias + topk-weight
           scale + scatter-add combine.
  device : core e owns expert e. Both GEMMs run as fp8e4m3 DoubleRow
           matmuls (2 MACs/PE-cell/cycle) with a hi+lo error-compensation
           scheme:
             main MM (per 128-chunk c):  (Whi_c, Whi_c) x (x_hi_c, x_lo_c)
                 -> Whi.(x_hi + x_lo)   [activation quantization ~exact]
             corr MM (per chunk pair):  (Wlo_c0, Wlo_c1) x (x_hi_c0, x_hi_c1)
                 -> restores the weight lo plane, leaving only the
                    O(eps^2) Wlo.x_lo term
           fc2 corrects all 16 pairs; fc1 skips its (4x costlier per
           variance) corrections entirely: measured max-rel-err 1.78e-2
           (< 2e-2 gate, deterministic) at minimum PE cost.
           Weights are pre-scaled (x32 fc1 / x64 fc2) so the hi plane
           avoids the fp8 subnormal range; the scale is undone in the
           gelu activation (1/32) and in the host combine (1/64).
  engines: PE matmuls; ACT gelu(psum)->bf16 staging; gpsimd quantizes the
           fp8 hi plane; DVE computes the lo residual + drains psum;
           SP carries yn/w1/w2 DMAs, ACT queue carries b1 + output DMAs.
"""

import sys

for _p in ("/opt/trn_rl_repo", "/root/.axon_site/_ro/trn_rl_repo"):
    if _p not in sys.path:
        sys.path.append(_p)

import numpy as np
import ml_dtypes

import concourse.bass as bass
import concourse.bacc as bacc
import concourse.tile as tile
from concourse import mybir
from concourse.bass_utils import run_bass_kernel_spmd

F8 = ml_dtypes.float8_e4m3
BF16 = ml_dtypes.bfloat16

D = 1024          # embed dim
F = 4096          # hidden dim
N_EXP = 8         # experts == cores
BS = 8192         # tokens
K_TOK = 1024      # tokens kept per expert (BS * capacity / n_exp)
LN_EPS = 1e-5

P = 128
KD = D // P       # 8   d-chunks
KF = F // P       # 32  f-chunks
TH = 512          # moving free dim per matmul (one PSUM bank)
NT = K_TOK // TH  # 2   token halves
NC1_CORR = 0      # fc1: corrected chunk pairs (of KD//2 = 4)
NC2_CORR = 16     # fc2: corrected chunk pairs (of KF//2 = 16)
W1_SCALE = 32.0
W2_SCALE = 64.0

_NC_CACHE = {}
DR = mybir.MatmulPerfMode.DoubleRow


def _build_nc(debug=False, reps=1, warm=8, tsplit=True):
    nc = bacc.Bacc("TRN2", target_bir_lowering=False, debug=debug)
    f32 = mybir.dt.float32
    bf16 = mybir.dt.bfloat16
    fp8 = mybir.dt.float8e4

    # hi/lo token planes: row d holds (hi[d, :], lo[d, :])
    ynt = nc.dram_tensor("ynt", [D, 2 * K_TOK], fp8, kind="ExternalInput")
    # fc1 stripes: [f-block m][p_d, hi planes c=0..KD-1 then lo planes of
    # the corrected pairs] — uncorrected lo planes never ship
    W1C = KD + 2 * NC1_CORR
    w1s = nc.dram_tensor("w1s", [KF, P, W1C * P], fp8, kind="ExternalInput")
    # fc2 m-major: [d-block m][p_f, f-chunk c, plane hi/lo, p_d]
    w2s = nc.dram_tensor("w2s", [KD, P, KF * 2 * P], fp8, kind="ExternalInput")
    b1r = nc.dram_tensor("b1r", [P, KF], f32, kind="ExternalInput")
    ot = nc.dram_tensor("ot", [D, K_TOK], bf16, kind="ExternalOutput")

    GELU = mybir.ActivationFunctionType.Gelu_apprx_tanh

    with tile.TileContext(nc) as tc:
        with (
            tc.tile_pool(name="singles", bufs=1) as singles,
            tc.tile_pool(name="big", bufs=1) as big,
            tc.tile_pool(name="w1p", bufs=3) as w1p,
            tc.tile_pool(name="hfp", bufs=4) as hfp,
            tc.tile_pool(name="outp", bufs=2) as outp,
            tc.tile_pool(name="psum", bufs=4, space="PSUM") as psum,
        ):
          for _rep in range(reps):
            # ---- PE warm-up: dummy matmuls on zeroed scratch burn the
            # p-state ramp (~3us at half rate) inside the initial DMA fill ----
            warm_w = singles.tile([P, P], bf16, name="warm_w")
            warm_x = singles.tile([P, 256], bf16, name="warm_x")
            nc.vector.memset(warm_w, 0.0)
            nc.vector.memset(warm_x, 0.0)
            if warm:
                warm_ps = psum.tile([P, 256], f32, tag="ps", name="warm_ps")
                for i in range(warm):
                    nc.tensor.matmul(warm_ps, warm_w, warm_x,
                                     start=(i == 0), stop=(i == warm - 1))

            # ---- fc1 stripe 0 + first token chunks race in front on
            # separate queues so the first real matmul starts ~3us in ----
            yn_sb = big.tile([P, KD, 2, K_TOK], fp8)
            w1_pre = w1p.tile([P, 4, W1C, P], fp8, name="w1pre")
            def yn_load(c0, c1):
                nc.sync.dma_start(
                    out=yn_sb[:, c0:c1, :, :],
                    in_=ynt[c0 * P:c1 * P, :].rearrange(
                        "(c p) k -> p c k", c=c1 - c0))
            yn_load(0, 2)
            nc.sync.dma_start(out=w1_pre[:, 0:1],
                              in_=w1s[0:1].rearrange("g p x -> p g x"))
            yn_load(2, 4)
            nc.sync.dma_start(out=w1_pre[:, 1:4],
                              in_=w1s[1:4].rearrange("g p x -> p g x"))
            yn_load(4, 6)
            yn_load(6, 8)

            b1_sb = singles.tile([P, KF], f32)
            nc.scalar.dma_start(out=b1_sb, in_=b1r[:, :])

            # ---- fc1: h = gelu(W1.T @ yn / 32 + b1) -> fp8 hi/lo planes ----
            h_sb = big.tile([P, KF, 2, K_TOK], fp8)
            w2_sb = big.tile([P, KD, KF, 2, P], fp8)
            w2_loaded = 0
            assert NC1_CORR == 0

            def fc1_post(m, ps):
                # single gelu -> bf16 staging; hi plane quantize on gpsimd;
                # lo plane residual on DVE
                hf = hfp.tile([P, K_TOK], bf16)
                nc.scalar.activation(hf, ps, GELU,
                                     bias=b1_sb[:, m:m + 1], scale=1.0 / W1_SCALE)
                hhi = h_sb[:, m, 0, :]
                nc.gpsimd.tensor_copy(out=hhi, in_=hf)
                nc.vector.tensor_tensor(
                    out=h_sb[:, m, 1, :], in0=hf, in1=hhi,
                    op=mybir.AluOpType.subtract,
                )

            def fc1_mm(ps, w1t, c):
                wmain = w1t[:, c:c + 1, :].broadcast_to([P, 2, P])
                for t in range(NT):
                    nc.tensor.matmul(
                        ps[:, t * TH:(t + 1) * TH], wmain,
                        yn_sb[:, c, :, t * TH:(t + 1) * TH],
                        start=(c == 0), stop=(c == KD - 1), perf_mode=DR,
                    )

            # first 4 f-blocks run chunk-pair-major, interleaved across
            # blocks, so early matmuls execute on already-arrived yn pairs
            # while the later pairs are still streaming in
            ps03 = [psum.tile([P, 2 * TH], f32, tag="ps", name=f"ps1_{m}")
                    for m in range(4)]
            for cp in range(KD // 2):
                for m in range(4):
                    for c in (2 * cp, 2 * cp + 1):
                        fc1_mm(ps03[m], w1_pre[:, m], c)
            for m in range(4):
                fc1_post(m, ps03[m])

            for m in range(4, KF):
                if m % 4 == 0:
                    w1g = w1p.tile([P, 4, W1C, P], fp8)
                    nc.sync.dma_start(
                        out=w1g, in_=w1s[m:m + 4].rearrange("g p x -> p g x"))
                w1t = w1g[:, m % 4]
                ps = psum.tile([P, 2 * TH], f32, tag="ps", name=f"ps1_{m}")
                for c in range(KD):
                    fc1_mm(ps, w1t, c)
                fc1_post(m, ps)
                # interleave fc2 weight loads with the tail of the w1 stream
                if m >= 24 and w2_loaded < KD:
                    nc.sync.dma_start(out=w2_sb[:, w2_loaded], in_=w2s[w2_loaded])
                    w2_loaded += 1

            # ---- fc2: o = W2.T @ (h_hi + h_lo) / 64, fully corrected ----
            for m in range(KD):
                w2t = w2_sb[:, m]
                last = m == KD - 1
                o_t = outp.tile([P, K_TOK], bf16)
                if last and tsplit:
                    # token-half-major with separate psum banks: first half
                    # drains under the second half's matmuls, and the final
                    # half drains via ACT + SP (both idle) for a short tail
                    spans = [(0, TH), (TH, TH + 448), (TH + 448, 2 * TH)]
                    for si, (lo, hi) in enumerate(spans):
                        w = hi - lo
                        pss = psum.tile([P, w], f32, tag="ps", name=f"ps2L_{si}")
                        for c in range(KF):
                            wmain = w2t[:, c, 0:1, :].broadcast_to([P, 2, P])
                            nc.tensor.matmul(
                                pss, wmain, h_sb[:, c, :, lo:hi],
                                start=(c == 0), stop=False, perf_mode=DR,
                            )
                        for j in range(NC2_CORR):
                            wcorr = w2t[:, 2 * j:2 * j + 2, 1, :]
                            nc.tensor.matmul(
                                pss, wcorr, h_sb[:, 2 * j:2 * j + 2, 0, lo:hi],
                                start=False, stop=(j == NC2_CORR - 1), perf_mode=DR,
                            )
                        if si < 2:
                            nc.vector.tensor_copy(out=o_t[:, lo:hi], in_=pss)
                            nc.scalar.dma_start(
                                out=ot[m * P:(m + 1) * P, lo:hi], in_=o_t[:, lo:hi])
                        else:
                            # final 64 cols: ACT identity + SP DMA, both idle,
                            # so the exit tail is as short as possible
                            nc.scalar.activation(
                                o_t[:, lo:hi], pss,
                                mybir.ActivationFunctionType.Identity)
                            nc.sync.dma_start(
                                out=ot[m * P:(m + 1) * P, lo:hi], in_=o_t[:, lo:hi])
                else:
                    ps2 = psum.tile([P, 2 * TH], f32, tag="ps", name=f"ps2_{m}")
                    for c in range(KF):
                        wmain = w2t[:, c, 0:1, :].broadcast_to([P, 2, P])
                        for t in range(NT):
                            nc.tensor.matmul(
                                ps2[:, t * TH:(t + 1) * TH], wmain,
                                h_sb[:, c, :, t * TH:(t + 1) * TH],
                                start=(c == 0), stop=False, perf_mode=DR,
                            )
                    for j in range(NC2_CORR):
                        wcorr = w2t[:, 2 * j:2 * j + 2, 1, :]
                        for t in range(NT):
                            nc.tensor.matmul(
                                ps2[:, t * TH:(t + 1) * TH], wcorr,
                                h_sb[:, 2 * j:2 * j + 2, 0, t * TH:(t + 1) * TH],
                                start=False, stop=(j == NC2_CORR - 1), perf_mode=DR,
                            )
                    nc.vector.tensor_copy(out=o_t, in_=ps2)
                    nc.scalar.dma_start(out=ot[m * P:(m + 1) * P, :], in_=o_t)

    nc.compile()
    return nc


def get_nc():
    if "nc" not in _NC_CACHE:
        _NC_CACHE["nc"] = _build_nc()
    return _NC_CACHE["nc"]


def _gate_topk(xf32, gate_w):
    """Replicates the reference gating bit-exactly (same jnp ops, same backend)."""
    import jax
    import jax.numpy as jnp

    xf = jnp.asarray(xf32)
    gw = jnp.asarray(np.asarray(gate_w, dtype=np.float32))
    scores = xf @ gw.T
    scores = (jnp.tanh(scores) + 1.0) * 0.5
    vals, idx = jax.lax.top_k(scores.T, K_TOK)   # (n, k)
    return np.asarray(vals), np.asarray(idx)


def _q8(a):
    return np.clip(a, -240.0, 240.0).astype(F8)


def _hilo(a):
    """fp8 hi/lo decomposition: a ~= hi + lo with O(eps^2) residual."""
    hi = _q8(a)
    lo = _q8(a - hi.astype(np.float32))
    return hi, lo


def kernel(x, gate_w, ln_gamma, ln_beta, fc1s, b1s, fc2s, b2s):
    x = np.asarray(x, dtype=np.float32)
    gate_w = np.asarray(gate_w, dtype=np.float32)
    ln_gamma = np.asarray(ln_gamma, dtype=np.float32)
    ln_beta = np.asarray(ln_beta, dtype=np.float32)
    fc1s = np.asarray(fc1s, dtype=np.float32)
    b1s = np.asarray(b1s, dtype=np.float32)
    fc2s = np.asarray(fc2s, dtype=np.float32)
    b2s = np.asarray(b2s, dtype=np.float32)

    og_shape = x.shape
    xf = x.reshape(-1, D)
    vals, idx = _gate_topk(xf, gate_w)

    np_inputs = {"ln_gamma": ln_gamma, "ln_beta": ln_beta,
                 "fc1s": fc1s, "b1s": b1s, "fc2s": fc2s, "b2s": b2s}
    in_maps = build_in_maps(np_inputs, xf, vals, idx)

    nc = get_nc()
    res = run_bass_kernel_spmd(nc, in_maps, core_ids=list(range(N_EXP)))

    out = xf.copy()
    for e in range(N_EXP):
        o_e = np.asarray(res.results[e]["ot"]).astype(np.float32).T
        o_e *= 1.0 / W2_SCALE
        o_e += b2s[e]
        out[idx[e]] += o_e * vals[e][:, None]
    return out.reshape(og_shape)


def build_in_maps(np_inputs, xf, vals, idx):
    gam = np_inputs["ln_gamma"]
    bet = np_inputs["ln_beta"]
    maps = []
    for e in range(N_EXP):
        y_e = xf[idx[e]]                                   # (k, d) f32
        mu = y_e.mean(axis=1, keepdims=True)
        var = y_e.var(axis=1, keepdims=True)
        yn = (y_e - mu) / np.sqrt(var + LN_EPS) * gam + bet
        yh, yl = _hilo(yn)                                 # (k, d) each
        ynt = np.stack([yh.T, yl.T], axis=1)               # (d, 2, k)

        w1h, w1l = _hilo(np_inputs["fc1s"][e] * W1_SCALE)  # (F, D)
        w1h = w1h.reshape(KF, P, KD, P).transpose(0, 3, 2, 1)   # kf, p_d, kd, p_f
        w1l = w1l.reshape(KF, P, KD, P).transpose(0, 3, 2, 1)
        w1 = np.concatenate([w1h, w1l[:, :, 0:2 * NC1_CORR]], axis=2)

        w2h, w2l = _hilo(np_inputs["fc2s"][e] * W2_SCALE)  # (D, F)
        w2 = np.stack([w2h, w2l], axis=0).reshape(2, KD, P, KF, P)
        w2 = w2.transpose(1, 4, 3, 0, 2)                   # kd, p_f, kf, plane, p_d

        maps.append({
            "ynt": np.ascontiguousarray(ynt).reshape(D, 2 * K_TOK),
            "w1s": np.ascontiguousarray(w1).reshape(KF, P, (KD + 2 * NC1_CORR) * P),
            "w2s": np.ascontiguousarray(w2).reshape(KD, P, KF * 2 * P),
            "b1r": np.ascontiguousarray(np_inputs["b1s"][e].reshape(KF, P).T),
        })
    return maps
